# revision 1
# baseline (speedup 1.0000x reference)
"""Trainium2 Bass kernel for BuNN (nn_BuNN_10797547782311).

Strategy: row-shard L (and node features) over 8 NeuronCores. Each layer's
Taylor heat-diffusion loop streams the local [N/8, N] block of L (pre-cast
to bf16, pre-transposed on host so DMA is contiguous) through the tensor
engine against a stationary, replicated copy of the current Taylor term.
The new term is produced feature-major in PSUM, accumulated into the layer
result in fp32, cast+scaled to bf16, transposed back to node-major on the
PE, and AllGathered in 4 pipelined chunks so the next step can start as
soon as the first chunk lands. All node-parallel ops (phi MLP, rotations,
linear transforms, GELU) run feature-major ([feature, node] tiles) with a
td-permutation that places rotation x/y components in partition halves.
"""

import os
import sys
import types

import numpy as np
import ml_dtypes

import concourse.bacc as bacc
import concourse.tile as tile
from concourse import mybir
from concourse.bass_utils import run_bass_kernel_spmd
from concourse.bass import ds
from concourse.masks import make_identity

# Problem config (hardcoded per contest rules)
N, D_IN, D_OUT = 16384, 128, 40
B = 32
TD = 2 * B          # 64
HID = 2 * B         # 64
NL = 4              # layers
K = 8               # Taylor steps
M = 8               # cores
R = N // M          # 2048 rows per core
NCH = 4             # AllGather chunks per step
CH = R // NCH       # 512 rows per chunk
AB = CH // 128      # 4 k-blocks per batched DMA

f32 = mybir.dt.float32
bf16 = mybir.dt.bfloat16
BF = ml_dtypes.bfloat16

_CACHE = {}


def _install_ntff_shim():
    try:
        from antenv.axon_hooks import get_axon_ntff_profile_hook  # noqa: F401
    except ImportError:
        try:
            from trn_agent_boot.trn_boot import _ntff_profile_via_ctypes

            _hook = _ntff_profile_via_ctypes("/opt/axon/libaxon_pjrt.so")
            _m = types.ModuleType("antenv.axon_hooks")
            _m.get_axon_ntff_profile_hook = lambda: _hook
            _m.set_axon_ntff_profile_hook = lambda h: None
            sys.modules["antenv.axon_hooks"] = _m
        except Exception:
            pass


def _build():
    nc = bacc.Bacc(None, target_bir_lowering=False, debug=False, num_devices=M)

    # ---- per-core inputs (host pre-transformed)
    xT_d = nc.dram_tensor("xT", [D_IN, R], f32, kind="ExternalInput")
    Lt_d = nc.dram_tensor("Lt", [NCH * M, 128, AB * R], bf16, kind="ExternalInput")
    embWt_d = nc.dram_tensor("embWt", [D_IN, TD], f32, kind="ExternalInput")
    embB_d = nc.dram_tensor("embB", [TD, 1], f32, kind="ExternalInput")
    w1_d = nc.dram_tensor("w1", [NL, TD, HID], f32, kind="ExternalInput")
    b1_d = nc.dram_tensor("b1", [NL, HID, 1], f32, kind="ExternalInput")
    w2_d = nc.dram_tensor("w2", [NL, HID, TD], f32, kind="ExternalInput")
    b2s_d = nc.dram_tensor("b2s", [NL, TD, 1], f32, kind="ExternalInput")
    b2c_d = nc.dram_tensor("b2c", [NL, TD, 1], f32, kind="ExternalInput")
    ltw_d = nc.dram_tensor("ltw", [NL, TD, TD], f32, kind="ExternalInput")
    ltb_d = nc.dram_tensor("ltb", [NL, TD, 1], f32, kind="ExternalInput")
    outw_d = nc.dram_tensor("outw", [TD, D_OUT], f32, kind="ExternalInput")
    outb_d = nc.dram_tensor("outb", [D_OUT, 1], f32, kind="ExternalInput")

    outT_d = nc.dram_tensor("outT", [D_OUT, R], f32, kind="ExternalOutput")

    # ---- collective buffers: per chunk, ping-pong by step parity
    loc_d = [nc.dram_tensor(f"loc{c}", [128, AB * TD], bf16) for c in range(NCH)]
    full_d = [
        [
            nc.dram_tensor(f"full{c}_{p}", [M * 128, AB * TD], bf16, addr_space="Shared")
            for p in range(2)
        ]
        for c in range(NCH)
    ]
    RG = [list(range(M))]

    with tile.TileContext(nc) as tc:
        with (
            tc.tile_pool(name="lpool", bufs=8) as lpool,
            tc.tile_pool(name="tpool", bufs=4) as tpool,
            tc.tile_pool(name="mmps", bufs=1, space="PSUM") as mmps,
            tc.tile_pool(name="trp", bufs=2, space="PSUM") as trp,
            tc.tile_pool(name="work", bufs=2) as work,
            tc.tile_pool(name="packp", bufs=8) as packp,
            tc.tile_pool(name="wk1", bufs=1) as wk1,
            tc.tile_pool(name="sg", bufs=1) as sg,
        ):
            # ---- persistent SBUF state
            ident = sg.tile([TD, TD], bf16)
            make_identity(nc, ident[:])
            h_sb = sg.tile([TD, R], f32)
            res_sb = sg.tile([TD, R], f32)
            c2_sb = sg.tile([TD, R], f32)
            ssgn_sb = sg.tile([TD, R], f32)
            tbf_sb = sg.tile([TD, R], bf16)

            # weights resident in SBUF
            embWt = sg.tile([D_IN, TD], f32)
            nc.sync.dma_start(out=embWt[:], in_=embWt_d[:, :])
            embB = sg.tile([TD, 1], f32)
            nc.sync.dma_start(out=embB[:], in_=embB_d[:, :])
            w1 = [sg.tile([TD, HID], f32, tag=f"w1_{i}", name=f"w1_{i}") for i in range(NL)]
            b1 = [sg.tile([HID, 1], f32, tag=f"b1_{i}", name=f"b1_{i}") for i in range(NL)]
            w2 = [sg.tile([HID, TD], f32, tag=f"w2_{i}", name=f"w2_{i}") for i in range(NL)]
            b2s = [sg.tile([TD, 1], f32, tag=f"b2s_{i}", name=f"b2s_{i}") for i in range(NL)]
            b2c = [sg.tile([TD, 1], f32, tag=f"b2c_{i}", name=f"b2c_{i}") for i in range(NL)]
            ltw = [sg.tile([TD, TD], f32, tag=f"ltw_{i}", name=f"ltw_{i}") for i in range(NL)]
            ltb = [sg.tile([TD, 1], f32, tag=f"ltb_{i}", name=f"ltb_{i}") for i in range(NL)]
            for i in range(NL):
                nc.sync.dma_start(out=w1[i][:], in_=w1_d[i, :, :])
                nc.sync.dma_start(out=b1[i][:], in_=b1_d[i, :, :])
                nc.sync.dma_start(out=w2[i][:], in_=w2_d[i, :, :])
                nc.sync.dma_start(out=b2s[i][:], in_=b2s_d[i, :, :])
                nc.sync.dma_start(out=b2c[i][:], in_=b2c_d[i, :, :])
                nc.sync.dma_start(out=ltw[i][:], in_=ltw_d[i, :, :])
                nc.sync.dma_start(out=ltb[i][:], in_=ltb_d[i, :, :])
            outw = sg.tile([TD, D_OUT], f32)
            nc.sync.dma_start(out=outw[:], in_=outw_d[:, :])
            outb = sg.tile([D_OUT, 1], f32)
            nc.sync.dma_start(out=outb[:], in_=outb_d[:, :])

            pid = nc.gpsimd.partition_id()
            qrow = [
                nc.gpsimd.snap(((pid + si) % M) * 128) for si in range(1, M)
            ]

            xT = wk1.tile([D_IN, R], f32, tag="g", name="xT")
            nc.sync.dma_start(out=xT[:], in_=xT_d[:, :])

            # ---- embedding: h = emb(x)
            ps = mmps.tile([TD, R], f32, tag="mmps")
            for n in range(R // 512):
                nc.tensor.matmul(
                    ps[:, n * 512 : (n + 1) * 512],
                    embWt[:],
                    xT[:, n * 512 : (n + 1) * 512],
                    start=True,
                    stop=True,
                )
            nc.vector.tensor_scalar_add(h_sb[:], ps[:], embB[:])

            def cast_and_send(src_psum_or_sb, scale, parity, from_psum):
                """Cast src*scale to bf16 into tbf_sb, then per chunk:
                PE-transpose to node-major, DMA to loc, AllGather.
                Returns the node-major SBUF pack tiles (self lhsT source)."""
                packs = []
                for c in range(NCH):
                    sl = slice(c * CH, (c + 1) * CH)
                    nc.scalar.activation(
                        tbf_sb[:, sl],
                        src_psum_or_sb[:, sl],
                        mybir.ActivationFunctionType.Copy,
                        scale=scale,
                    )
                    pack = packp.tile([128, AB * TD], bf16, tag="pack")
                    for j in range(AB):
                        trps = trp.tile([128, TD], bf16, tag="trp")
                        nc.tensor.transpose(
                            trps[:],
                            tbf_sb[:, c * CH + j * 128 : c * CH + (j + 1) * 128],
                            ident[:],
                        )
                        nc.vector.tensor_copy(pack[:, j * TD : (j + 1) * TD], trps[:])
                    packs.append(pack)
                    nc.gpsimd.dma_start(out=loc_d[c][:, :], in_=pack[:])
                    nc.gpsimd.collective_compute(
                        "AllGather",
                        mybir.AluOpType.bypass,
                        replica_groups=RG,
                        ins=[loc_d[c][:, :]],
                        outs=[full_d[c][parity][:, :]],
                    )
                return packs

            def taylor_step(k, parity, packs):
                """psum_acc = contraction of the (pre-scaled) term with Lt.
                Self chunks come from the SBUF pack tiles (no AG wait) and
                run first; remote rank blocks stream from the AllGather
                output at rank-relative dynamic offsets. Host packs Lt tiles
                in matching order. Returns the psum holding term_k."""
                acc = mmps.tile([TD, R], f32, tag="mmps")

                def mm16(lhsT_tile, lt_tile, start, stop):
                    for j in range(AB):
                        for n in range(R // 512):
                            nc.tensor.matmul(
                                acc[:, n * 512 : (n + 1) * 512],
                                lhsT_tile[:, j * TD : (j + 1) * TD],
                                lt_tile[:, j * R + n * 512 : j * R + (n + 1) * 512],
                                start=start and j == 0,
                                stop=stop and j == AB - 1,
                            )

                # self chunks first: lhsT straight from SBUF packs
                for c in range(NCH):
                    lt = lpool.tile([128, AB * R], bf16, tag="lt")
                    nc.sync.dma_start(out=lt[:], in_=Lt_d[c, :, :])
                    mm16(packs[c], lt, start=(c == 0), stop=False)
                # remote rank blocks, rank-relative order
                for c in range(NCH):
                    for si in range(1, M):
                        u = NCH + c * (M - 1) + (si - 1)
                        tt = tpool.tile([128, AB * TD], bf16, tag="tt")
                        nc.gpsimd.dma_start(
                            out=tt[:],
                            in_=full_d[c][parity][ds(qrow[si - 1], 128), :],
                        )
                        lt = lpool.tile([128, AB * R], bf16, tag="lt")
                        nc.sync.dma_start(out=lt[:], in_=Lt_d[u, :, :])
                        mm16(
                            tt,
                            lt,
                            start=False,
                            stop=(c == NCH - 1 and si == M - 1),
                        )
                return acc

            for i in range(NL):
                # ---- phi MLP -> signed duplicated angles -> sin/cos
                ps1 = mmps.tile([HID, R], f32, tag="mmps")
                for n in range(R // 512):
                    nc.tensor.matmul(
                        ps1[:, n * 512 : (n + 1) * 512],
                        w1[i][:],
                        h_sb[:, n * 512 : (n + 1) * 512],
                        start=True,
                        stop=True,
                    )
                g_sb = wk1.tile([HID, R], f32, tag="g")
                nc.scalar.activation(
                    g_sb[:], ps1[:], mybir.ActivationFunctionType.Gelu, bias=b1[i][:]
                )
                ps2 = mmps.tile([TD, R], f32, tag="mmps")
                for n in range(R // 512):
                    nc.tensor.matmul(
                        ps2[:, n * 512 : (n + 1) * 512],
                        w2[i][:],
                        g_sb[:, n * 512 : (n + 1) * 512],
                        start=True,
                        stop=True,
                    )
                nc.scalar.activation(
                    ssgn_sb[:], ps2[:], mybir.ActivationFunctionType.Sin, bias=b2s[i][:]
                )
                nc.scalar.activation(
                    c2_sb[:], ps2[:], mybir.ActivationFunctionType.Sin, bias=b2c[i][:]
                )

                # ---- rotate into bundle frame:
                # row b (<32):  c*x - s*y ; row 32+b: c*y + s*x
                swap = wk1.tile([TD, R], f32, tag="swap")
                nc.vector.tensor_copy(swap[0:B, :], h_sb[B:TD, :])
                nc.vector.tensor_copy(swap[B:TD, :], h_sb[0:B, :])
                rot = wk1.tile([TD, R], f32, tag="rot")
                nc.vector.tensor_mul(rot[:], c2_sb[:], h_sb[:])
                tmp = wk1.tile([TD, R], f32, tag="tmp")
                nc.vector.tensor_mul(tmp[:], ssgn_sb[:], swap[:])
                nc.vector.tensor_add(rot[:], rot[:], tmp[:])

                # ---- linear transform H = lt(rot)
                psH = mmps.tile([TD, R], f32, tag="mmps")
                for n in range(R // 512):
                    nc.tensor.matmul(
                        psH[:, n * 512 : (n + 1) * 512],
                        ltw[i][:],
                        rot[:, n * 512 : (n + 1) * 512],
                        start=True,
                        stop=True,
                    )
                nc.vector.tensor_scalar_add(res_sb[:], psH[:], ltb[i][:])

                # term_0 = H; stationary operand for step 1 is -H
                packs = cast_and_send(res_sb, -1.0, 0, from_psum=False)

                # ---- Taylor diffusion
                for k in range(1, K + 1):
                    acc = taylor_step(k, (k - 1) % 2, packs)
                    if k < K:
                        stage = wk1.tile(
                            [TD, R], f32, tag="tmp", name=f"stage_{i}_{k}"
                        )
                        nc.scalar.activation(
                            stage[:], acc[:], mybir.ActivationFunctionType.Copy
                        )
                        packs = cast_and_send(
                            stage, -1.0 / (k + 1), k % 2, from_psum=False
                        )
                        nc.vector.tensor_add(res_sb[:], res_sb[:], stage[:])
                    else:
                        nc.vector.tensor_add(res_sb[:], res_sb[:], acc[:])

                # ---- rotate back, gelu, residual
                swap2 = wk1.tile([TD, R], f32, tag="swap")
                nc.vector.tensor_copy(swap2[0:B, :], res_sb[B:TD, :])
                nc.vector.tensor_copy(swap2[B:TD, :], res_sb[0:B, :])
                rot2 = wk1.tile([TD, R], f32, tag="rot")
                nc.vector.tensor_mul(rot2[:], c2_sb[:], res_sb[:])
                tmp2 = wk1.tile([TD, R], f32, tag="tmp")
                nc.vector.tensor_mul(tmp2[:], ssgn_sb[:], swap2[:])
                nc.vector.tensor_sub(rot2[:], rot2[:], tmp2[:])
                g2 = wk1.tile([TD, R], f32, tag="g")
                nc.scalar.activation(
                    g2[:], rot2[:], mybir.ActivationFunctionType.Gelu
                )
                nc.vector.tensor_add(h_sb[:], h_sb[:], g2[:])

            # ---- output projection
            pso = mmps.tile([D_OUT, R], f32, tag="mmps")
            for n in range(R // 512):
                nc.tensor.matmul(
                    pso[:, n * 512 : (n + 1) * 512],
                    outw[:],
                    h_sb[:, n * 512 : (n + 1) * 512],
                    start=True,
                    stop=True,
                )
            o_sb = wk1.tile([D_OUT, R], f32, tag="tmp")
            nc.vector.tensor_scalar_add(o_sb[:], pso[:], outb[:])
            nc.sync.dma_start(out=outT_d[:, :], in_=o_sb[:])

    nc.compile()
    return nc


def kernel(**inputs):
    x = np.asarray(inputs["x"], dtype=np.float32)
    L = np.asarray(inputs["L"], dtype=np.float32)
    emb_W = np.asarray(inputs["emb_W"], dtype=np.float32)
    emb_b = np.asarray(inputs["emb_b"], dtype=np.float32)
    phi_W1 = np.asarray(inputs["phi_W1"], dtype=np.float32)
    phi_b1 = np.asarray(inputs["phi_b1"], dtype=np.float32)
    phi_W2 = np.asarray(inputs["phi_W2"], dtype=np.float32)
    phi_b2 = np.asarray(inputs["phi_b2"], dtype=np.float32)
    lt_W = np.asarray(inputs["lt_W"], dtype=np.float32)
    lt_b = np.asarray(inputs["lt_b"], dtype=np.float32)
    out_W = np.asarray(inputs["out_W"], dtype=np.float32)
    out_b = np.asarray(inputs["out_b"], dtype=np.float32)

    perm = np.concatenate([np.arange(0, TD, 2), np.arange(1, TD, 2)])

    embWt = np.ascontiguousarray(emb_W.T[:, perm])
    embB = np.ascontiguousarray(emb_b[perm][:, None])
    w1 = np.ascontiguousarray(
        np.stack([phi_W1[i].T[perm, :] for i in range(NL)])
    )
    b1 = np.ascontiguousarray(phi_b1[:, :, None])
    w2 = np.ascontiguousarray(
        np.stack(
            [np.concatenate([-phi_W2[i].T, phi_W2[i].T], axis=1) for i in range(NL)]
        )
    )
    b2s = np.ascontiguousarray(
        np.stack([np.concatenate([-phi_b2[i], phi_b2[i]])[:, None] for i in range(NL)])
    )
    b2c = (b2s + np.float32(np.pi / 2)).astype(np.float32)
    ltw = np.ascontiguousarray(
        np.stack([lt_W[i].T[perm][:, perm] for i in range(NL)])
    )
    ltb = np.ascontiguousarray(
        np.stack([lt_b[i][perm][:, None] for i in range(NL)])
    )
    outw = np.ascontiguousarray(out_W.T[perm, :])
    outb = np.ascontiguousarray(out_b[:, None])

    Lbf = L.astype(BF)

    def _tile_lt(Lbf_, c_):
        # LtC[k, n] = L[c_*R + n, k]. Tile order matches kernel consumption:
        # u=0..3 -> (chunk u, self rank); u>=4 -> chunk (u-4)//7, rank
        # (c_ + 1 + (u-4)%7) % 8. Each tile: rows q*R + ch*CH + j*128 + p
        # -> [u][p][j*R + n], contiguous per partition.
        LtC = np.ascontiguousarray(Lbf_[c_ * R : (c_ + 1) * R].T)  # [N, R]
        out = np.empty((NCH * M, 128, AB * R), dtype=BF)

        def put(u, ch, q):
            blk = LtC[q * R + ch * CH : q * R + (ch + 1) * CH]  # [512, R]
            out[u] = (
                blk.reshape(AB, 128, R).transpose(1, 0, 2).reshape(128, AB * R)
            )

        for ch in range(NCH):
            put(ch, ch, c_)
        for ch in range(NCH):
            for si in range(1, M):
                put(NCH + ch * (M - 1) + (si - 1), ch, (c_ + si) % M)
        return out

    shared = {
        "embWt": embWt, "embB": embB, "w1": w1, "b1": b1, "w2": w2,
        "b2s": b2s, "b2c": b2c, "ltw": ltw, "ltb": ltb,
        "outw": outw, "outb": outb,
    }
    in_maps = []
    for c in range(M):
        in_maps.append(
            {
                "xT": np.ascontiguousarray(x[c * R : (c + 1) * R].T),
                "Lt": _tile_lt(Lbf, c),
                **shared,
            }
        )

    if "nc" not in _CACHE:
        _CACHE["nc"] = _build()
    nc = _CACHE["nc"]

    trace = bool(os.environ.get("BUNN_TRACE"))
    if trace:
        _install_ntff_shim()
    res = run_bass_kernel_spmd(nc, in_maps, list(range(M)), trace=trace)
    if trace and res.exec_time_ns is not None:
        print(f"HW exec time: {res.exec_time_ns} ns")
        _CACHE["exec_time_ns"] = res.exec_time_ns

    out = np.empty((N, D_OUT), dtype=np.float32)
    for c in range(M):
        out[c * R : (c + 1) * R, :] = res.results[c]["outT"].T
    return out

